# revision 1
# baseline (speedup 1.0000x reference)
"""Distributed Trainium2 (Bass/Tile) kernel for AdaptiveGCNLayer.

Reference semantics (N=4096 nodes, C=512 channels):
    adj   = x @ W_adj @ x.T + I                      [N, N]
    adj   = d^-1/2 * adj * d^-1/2   (row sums d)     -- values then DISCARDED:
    A     = (adj != 0) with forced unit diagonal     (dense_to_sparse keeps only
                                                      the nonzero pattern)
    deg   = A.sum(1); dis = deg^-1/2 (0 if deg<=0)
    out   = (dis[:,None] * A * dis[None,:]) @ (x @ W_gcn) + b

Multiplying a row/column by a nonzero (or NaN/inf) scalar never changes the
!=0 pattern, so A == (x @ W_adj @ x.T != 0) except on the measure-zero event
of an exactly-zero entry; the first normalization therefore doesn't need to
be materialized.  deg >= 1 always (forced diagonal), so the deg>0 guard is
moot.

Sharding (8 cores, 1-D node partition, R=512 rows each): core i computes the
adjacency block for its rows in TRANSPOSED layout adjT [N, R] (so the block
is directly usable as the stationary operand of the final aggregation),
masks it to {0,1} bf16, reduces mask -> deg for its rows, AllGathers deg
(the "column degree" exchange), scales the AllGathered xg = x @ W_gcn by
dis, and computes out_rows = dis_r * (A_rows @ (dis * xg)) + b.

Compute dtype bf16 (matmuls; fp32 PSUM accumulation): the output only
depends on the adjacency through its nonzero pattern, which is insensitive
to rounding, and the final averaging keeps the result within ~1e-3 relative
of the f32 reference.

All eight cores run one SPMD NEFF via run_bass_kernel_spmd; per-core data
(own x^T column slice) is delivered through the per-core input maps.
"""

import numpy as np
import ml_dtypes

from concourse import bacc, mybir, tile
from concourse.bass_utils import run_bass_kernel_spmd

N_CORES = 8
N = 4096               # nodes
C = 512                # channels (C_IN == C_OUT)
R = N // N_CORES       # 512 rows per core
P = 128                # SBUF partitions
KT = C // P            # 4 contraction tiles
NT = N // P            # 32 node tiles
MT = R // P            # 4 row tiles per core

F32 = mybir.dt.float32
BF16 = mybir.dt.bfloat16
BF = ml_dtypes.bfloat16

_cache = {}


def _build():
    nc = bacc.Bacc("TRN2", target_bir_lowering=False, debug=False,
                   num_devices=N_CORES)

    xT = nc.dram_tensor("xT", [C, N], BF16, kind="ExternalInput")      # x^T, full
    xTs = nc.dram_tensor("xTs", [C, R], BF16, kind="ExternalInput")    # own cols of x^T
    adjW = nc.dram_tensor("adjW", [C, C], BF16, kind="ExternalInput")
    gcnW = nc.dram_tensor("gcnW", [C, C], BF16, kind="ExternalInput")
    bias = nc.dram_tensor("bias", [1, C], BF16, kind="ExternalInput")
    out = nc.dram_tensor("out", [R, C], F32, kind="ExternalOutput")

    rg = [list(range(N_CORES))]

    with tile.TileContext(nc) as tc:
        with (
            tc.tile_pool(name="sb", bufs=1) as sb,
            tc.tile_pool(name="sbo", bufs=2) as sbo,
            tc.tile_pool(name="dram", bufs=1, space="DRAM") as dram,
            tc.tile_pool(name="ps_a", bufs=2, space="PSUM") as ps_a,
            tc.tile_pool(name="ps_adj", bufs=3, space="PSUM") as ps_adj,
            tc.tile_pool(name="ps_deg", bufs=1, space="PSUM") as ps_deg,
            tc.tile_pool(name="ps_fin", bufs=2, space="PSUM") as ps_fin,
        ):
            # ---- input loads -------------------------------------------------
            adjW_sb = [sb.tile([P, C], BF16, name=f"adjW{k}", tag=f"adjW{k}") for k in range(KT)]
            xTs_sb = [sb.tile([P, R], BF16, name=f"xTs{k}", tag=f"xTs{k}") for k in range(KT)]
            gcnW_sb = [sb.tile([P, C], BF16, name=f"gcnW{k}", tag=f"gcnW{k}") for k in range(KT)]
            xT_sb = [sb.tile([P, N], BF16, name=f"xT{k}", tag=f"xT{k}") for k in range(KT)]
            bias_sb = sb.tile([1, C], BF16, name="bias_sb", tag="bias_sb")
            ones_col = sb.tile([P, 1], BF16, name="ones_col", tag="ones_col")

            for k in range(KT):
                nc.sync.dma_start(adjW_sb[k][:, :], adjW[P * k:P * (k + 1), :])
                nc.sync.dma_start(xTs_sb[k][:, :], xTs[P * k:P * (k + 1), :])
                nc.sync.dma_start(gcnW_sb[k][:, :], gcnW[P * k:P * (k + 1), :])
            for k in range(KT):
                nc.sync.dma_start(xT_sb[k][:, :], xT[P * k:P * (k + 1), :])
            nc.sync.dma_start(bias_sb[:, :], bias[:, :])
            nc.vector.memset(ones_col[:, :], 1.0)

            # ---- phase 1a: xwT[j, r] = sum_c W_adj[c, j] x^T[c, r] ----------
            xwT_sb = [sb.tile([P, R], BF16, name=f"xwT{j}", tag=f"xwT{j}") for j in range(KT)]
            for j in range(KT):
                pa = ps_a.tile([P, R], F32, name=f"psa{j}", tag="psa")
                for k in range(KT):
                    nc.tensor.matmul(pa[:, :],
                                     adjW_sb[k][:, P * j:P * (j + 1)],
                                     xTs_sb[k][:, :],
                                     start=(k == 0), stop=(k == KT - 1))
                nc.vector.tensor_copy(xwT_sb[j][:, :], pa[:, :])

            # ---- phase 1b: xg[r, f] = sum_c x[r, c] W_gcn[c, f] (own rows) --
            yb_in = dram.tile([R, C], BF16, name="yb_in", tag="yb_in")
            yb_out = dram.tile([N, C], BF16, addr_space="Shared", name="yb_out", tag="yb_out")
            xg_sb = [sb.tile([P, C], BF16, name=f"xg{m}", tag=f"xg{m}") for m in range(MT)]
            for m in range(MT):
                pa = ps_a.tile([P, C], F32, name=f"psg{m}", tag="psa")
                for k in range(KT):
                    nc.tensor.matmul(pa[:, :],
                                     xTs_sb[k][:, P * m:P * (m + 1)],
                                     gcnW_sb[k][:, :],
                                     start=(k == 0), stop=(k == KT - 1))
                nc.vector.tensor_copy(xg_sb[m][:, :], pa[:, :])
                nc.gpsimd.dma_start(yb_in[P * m:P * (m + 1), :], xg_sb[m][:, :])

            # AllGather xg -> y (unscaled yet); overlaps the adjacency phase.
            nc.gpsimd.collective_compute(
                "AllGather", mybir.AluOpType.bypass, replica_groups=rg,
                ins=[yb_in.opt()], outs=[yb_out.opt()])
            y_sb = [sb.tile([P, C], BF16, name=f"y{t}", tag=f"y{t}") for t in range(NT)]
            for t in range(NT):
                nc.sync.dma_start(y_sb[t][:, :], yb_out[P * t:P * (t + 1), :])

            # ---- phase 2: adjT tiles, mask, deg -----------------------------
            # adjT[n, r] = sum_j x^T[j, n] xwT[j, r]; mask = (adjT != 0);
            # deg[r] += ones(128)^T @ mask_tile  (accumulated over all 32 tiles)
            mask_sb = [sb.tile([P, R], BF16, name=f"mask{t}", tag=f"mask{t}") for t in range(NT)]
            pdeg = ps_deg.tile([1, R], F32, name="pdeg", tag="pdeg")
            for t in range(NT):
                pt = ps_adj.tile([P, R], F32, name=f"psadj{t}", tag="psadj")
                for k in range(KT):
                    nc.tensor.matmul(pt[:, :],
                                     xT_sb[k][:, P * t:P * (t + 1)],
                                     xwT_sb[k][:, :],
                                     start=(k == 0), stop=(k == KT - 1))
                nc.vector.tensor_scalar(mask_sb[t][:, :], pt[:, :], 0.0, None,
                                        mybir.AluOpType.not_equal)
                nc.tensor.matmul(pdeg[:, :], ones_col[:, :], mask_sb[t][:, :],
                                 start=(t == 0), stop=(t == NT - 1))

            deg_own = sb.tile([1, R], F32, name="deg_own", tag="deg_own")
            nc.vector.tensor_copy(deg_own[:, :], pdeg[:, :])

            # AllGather deg (the cross-core degree exchange).
            degb_in = dram.tile([R], F32, name="degb_in", tag="degb_in")
            degb_out = dram.tile([N], F32, addr_space="Shared", name="degb_out", tag="degb_out")
            nc.gpsimd.dma_start(degb_in[:], deg_own[:, :])
            nc.gpsimd.collective_compute(
                "AllGather", mybir.AluOpType.bypass, replica_groups=rg,
                ins=[degb_in.opt()], outs=[degb_out.opt()])

            # dis = deg^-1/2 in partition-major layouts (global and own-rows).
            deg_glob = sb.tile([P, NT], F32, name="deg_glob", tag="deg_glob")
            deg_ownp = sb.tile([P, MT], F32, name="deg_ownp", tag="deg_ownp")
            nc.sync.dma_start(deg_glob[:, :], degb_out.rearrange("(t p) -> p t", p=P))
            nc.sync.dma_start(deg_ownp[:, :], degb_in.rearrange("(t p) -> p t", p=P))
            dis_glob = sb.tile([P, NT], F32, name="dis_glob", tag="dis_glob")
            dis_own = sb.tile([P, MT], F32, name="dis_own", tag="dis_own")
            nc.vector.reciprocal(dis_glob[:, :], deg_glob[:, :])
            nc.scalar.sqrt(dis_glob[:, :], dis_glob[:, :])
            nc.vector.reciprocal(dis_own[:, :], deg_ownp[:, :])
            nc.scalar.sqrt(dis_own[:, :], dis_own[:, :])
            # sqrt(deg) row-vector: cancels the dis_r row scaling for the bias.
            invdis_row = sb.tile([1, R], BF16, name="invdis_row", tag="invdis_row")
            nc.scalar.sqrt(invdis_row[:, :], deg_own[:, :])

            # ---- phase 3: y *= dis; out_rows = dis_r * (A @ y) + b ----------
            for t in range(NT):
                nc.scalar.mul(y_sb[t][:, :], y_sb[t][:, :], dis_glob[:, t:t + 1])

            for m in range(MT):
                pf = ps_fin.tile([P, C], F32, name=f"psf{m}", tag="psf")
                for t in range(NT):
                    nc.tensor.matmul(pf[:, :],
                                     mask_sb[t][:, P * m:P * (m + 1)],
                                     y_sb[t][:, :],
                                     start=(t == 0), stop=False)
                # += sqrt(deg_r) (x) bias  — cancels against the dis_r scaling
                nc.tensor.matmul(pf[:, :],
                                 invdis_row[:, P * m:P * (m + 1)],
                                 bias_sb[:, :],
                                 start=False, stop=True)
                ot = sbo.tile([P, C], F32, name=f"outt{m}", tag="outt")
                nc.vector.tensor_scalar(ot[:, :], pf[:, :], dis_own[:, m:m + 1],
                                        None, mybir.AluOpType.mult)
                nc.sync.dma_start(out[P * m:P * (m + 1), :], ot[:, :])

    nc.compile()
    return nc


def _get_nc():
    if "nc" not in _cache:
        _cache["nc"] = _build()
    return _cache["nc"]


def _run(inputs, trace=False, trace_cores=None):
    x = np.asarray(inputs["x"], dtype=np.float32)
    adj_weight = np.asarray(inputs["adj_weight"], dtype=np.float32)
    gcn_weight = np.asarray(inputs["gcn_weight"], dtype=np.float32)
    gcn_bias = np.asarray(inputs["gcn_bias"], dtype=np.float32)

    xT = np.ascontiguousarray(x.T).astype(BF)          # [C, N]
    adjW = adj_weight.astype(BF)
    gcnW = gcn_weight.astype(BF)
    bias = gcn_bias.reshape(1, C).astype(BF)

    in_maps = []
    for i in range(N_CORES):
        in_maps.append({
            "xT": xT,
            "xTs": np.ascontiguousarray(xT[:, R * i:R * (i + 1)]),
            "adjW": adjW,
            "gcnW": gcnW,
            "bias": bias,
        })

    nc = _get_nc()
    res = run_bass_kernel_spmd(nc, in_maps, core_ids=list(range(N_CORES)),
                               trace=trace, trace_cores=trace_cores)
    full = np.concatenate([res.results[i]["out"] for i in range(N_CORES)], axis=0)
    return full, res


def kernel(**inputs):
    full, _ = _run(inputs, trace=False)
    return full


# revision 3
# speedup vs baseline: 1.1389x; 1.1389x over previous
"""Distributed Trainium2 (Bass/Tile) kernel for AdaptiveGCNLayer.

Reference semantics (N=4096 nodes, C=512 channels):
    adj   = x @ W_adj @ x.T + I                      [N, N]
    adj   = d^-1/2 * adj * d^-1/2   (row sums d)     -- values then DISCARDED:
    A     = (adj != 0) with forced unit diagonal     (dense_to_sparse keeps only
                                                      the nonzero pattern)
    deg   = A.sum(1); dis = deg^-1/2 (0 if deg<=0)
    out   = (dis[:,None] * A * dis[None,:]) @ (x @ W_gcn) + b

Multiplying a row/column by a nonzero (or NaN/inf) scalar never changes the
!=0 pattern, so A == (x @ W_adj @ x.T != 0) except on the measure-zero event
of an exactly-zero entry; the first normalization therefore doesn't need to
be materialized.  deg >= 1 always (forced diagonal), so the deg>0 guard is
moot.

Sharding (8 cores, 1-D node partition, R=512 rows each): core i computes the
adjacency block for its rows in TRANSPOSED layout adjT [N, R] (so the block
is directly usable as the stationary operand of the final aggregation),
masks it to {0,1} bf16, reduces mask -> deg for its rows, AllGathers deg
(the "column degree" exchange), scales the AllGathered xg = x @ W_gcn by
dis, and computes out_rows = dis_r * (A_rows @ (dis * xg)) + b.

Compute dtype bf16 (matmuls; fp32 PSUM accumulation): the output only
depends on the adjacency through its nonzero pattern, which is insensitive
to rounding, and the final averaging keeps the result within ~1e-3 relative
of the f32 reference.

All eight cores run one SPMD NEFF via run_bass_kernel_spmd; per-core data
(own x^T column slice) is delivered through the per-core input maps.
"""

import numpy as np
import ml_dtypes

from concourse import bacc, mybir, tile
from concourse.bass_utils import run_bass_kernel_spmd

N_CORES = 8
N = 4096               # nodes
C = 512                # channels (C_IN == C_OUT)
R = N // N_CORES       # 512 rows per core
P = 128                # SBUF partitions
KT = C // P            # 4 contraction tiles
NT = N // P            # 32 node tiles
MT = R // P            # 4 row tiles per core

F32 = mybir.dt.float32
BF16 = mybir.dt.bfloat16
BF = ml_dtypes.bfloat16

_cache = {}


def _build():
    nc = bacc.Bacc("TRN2", target_bir_lowering=False, debug=False,
                   num_devices=N_CORES)

    xT = nc.dram_tensor("xT", [C, N], BF16, kind="ExternalInput")      # x^T, full
    xTs = nc.dram_tensor("xTs", [C, R], BF16, kind="ExternalInput")    # own cols of x^T
    adjW = nc.dram_tensor("adjW", [C, C], BF16, kind="ExternalInput")
    gcnW = nc.dram_tensor("gcnW", [C, C], BF16, kind="ExternalInput")
    bias = nc.dram_tensor("bias", [1, C], BF16, kind="ExternalInput")
    out = nc.dram_tensor("out", [R, C], F32, kind="ExternalOutput")

    rg = [list(range(N_CORES))]

    with tile.TileContext(nc) as tc:
        with (
            tc.tile_pool(name="sb", bufs=1) as sb,
            tc.tile_pool(name="sbo", bufs=2) as sbo,
            tc.tile_pool(name="dram", bufs=1, space="DRAM") as dram,
            tc.tile_pool(name="ps_a", bufs=2, space="PSUM") as ps_a,
            tc.tile_pool(name="ps_adj", bufs=3, space="PSUM") as ps_adj,
            tc.tile_pool(name="ps_deg", bufs=1, space="PSUM") as ps_deg,
            tc.tile_pool(name="ps_fin", bufs=2, space="PSUM") as ps_fin,
        ):
            # ---- input loads -------------------------------------------------
            adjW_sb = [sb.tile([P, C], BF16, name=f"adjW{k}", tag=f"adjW{k}") for k in range(KT)]
            xTs_sb = [sb.tile([P, R], BF16, name=f"xTs{k}", tag=f"xTs{k}") for k in range(KT)]
            gcnW_sb = [sb.tile([P, C], BF16, name=f"gcnW{k}", tag=f"gcnW{k}") for k in range(KT)]
            xT_sb = [sb.tile([P, N], BF16, name=f"xT{k}", tag=f"xT{k}") for k in range(KT)]
            bias_sb = sb.tile([1, C], BF16, name="bias_sb", tag="bias_sb")
            ones_col = sb.tile([P, 1], BF16, name="ones_col", tag="ones_col")

            # xTs/gcnW first: phase 1b (xg -> AllGather trigger) is the
            # critical launch — the sooner every rank triggers its first
            # collective, the sooner the rank-skew barrier completes.
            for k in range(KT):
                nc.sync.dma_start(xTs_sb[k][:, :], xTs[P * k:P * (k + 1), :])
                nc.sync.dma_start(gcnW_sb[k][:, :], gcnW[P * k:P * (k + 1), :])
            for k in range(KT):
                nc.sync.dma_start(adjW_sb[k][:, :], adjW[P * k:P * (k + 1), :])
            for k in range(KT):
                nc.sync.dma_start(xT_sb[k][:, :], xT[P * k:P * (k + 1), :])
            nc.sync.dma_start(bias_sb[:, :], bias[:, :])
            nc.vector.memset(ones_col[:, :], 1.0)

            # ---- phase 1b: xg[r, f] = sum_c x[r, c] W_gcn[c, f] (own rows) --
            yb_in = dram.tile([R, C], BF16, name="yb_in", tag="yb_in")
            yb_out = dram.tile([N, C], BF16, addr_space="Shared", name="yb_out", tag="yb_out")
            xg_sb = [sb.tile([P, C], BF16, name=f"xg{m}", tag=f"xg{m}") for m in range(MT)]
            for m in range(MT):
                pa = ps_a.tile([P, C], F32, name=f"psg{m}", tag="psa")
                for k in range(KT):
                    nc.tensor.matmul(pa[:, :],
                                     xTs_sb[k][:, P * m:P * (m + 1)],
                                     gcnW_sb[k][:, :],
                                     start=(k == 0), stop=(k == KT - 1))
                nc.vector.tensor_copy(xg_sb[m][:, :], pa[:, :])
                nc.gpsimd.dma_start(yb_in[P * m:P * (m + 1), :], xg_sb[m][:, :])

            # AllGather xg -> y (unscaled yet); overlaps the adjacency phase.
            nc.gpsimd.collective_compute(
                "AllGather", mybir.AluOpType.bypass, replica_groups=rg,
                ins=[yb_in.opt()], outs=[yb_out.opt()])
            y_sb = [sb.tile([P, C], BF16, name=f"y{t}", tag=f"y{t}") for t in range(NT)]
            for t in range(NT):
                nc.sync.dma_start(y_sb[t][:, :], yb_out[P * t:P * (t + 1), :])

            # ---- phase 1a: xwT[j, r] = sum_c W_adj[c, j] x^T[c, r] ----------
            xwT_sb = [sb.tile([P, R], BF16, name=f"xwT{j}", tag=f"xwT{j}") for j in range(KT)]
            for j in range(KT):
                pa = ps_a.tile([P, R], F32, name=f"psa{j}", tag="psa")
                for k in range(KT):
                    nc.tensor.matmul(pa[:, :],
                                     adjW_sb[k][:, P * j:P * (j + 1)],
                                     xTs_sb[k][:, :],
                                     start=(k == 0), stop=(k == KT - 1))
                nc.vector.tensor_copy(xwT_sb[j][:, :], pa[:, :])

            # ---- phase 2: adjT tiles, mask, deg -----------------------------
            # adjT[n, r] = sum_j x^T[j, n] xwT[j, r]; mask = (adjT != 0);
            # deg[r] += ones(128)^T @ mask_tile  (accumulated over all 32 tiles)
            mask_sb = [sb.tile([P, R], BF16, name=f"mask{t}", tag=f"mask{t}") for t in range(NT)]
            pdeg = ps_deg.tile([1, R], F32, name="pdeg", tag="pdeg")
            for t in range(NT):
                pt = ps_adj.tile([P, R], F32, name=f"psadj{t}", tag="psadj")
                for k in range(KT):
                    nc.tensor.matmul(pt[:, :],
                                     xT_sb[k][:, P * t:P * (t + 1)],
                                     xwT_sb[k][:, :],
                                     start=(k == 0), stop=(k == KT - 1))
                nc.vector.tensor_scalar(mask_sb[t][:, :], pt[:, :], 0.0, None,
                                        mybir.AluOpType.not_equal)
                nc.tensor.matmul(pdeg[:, :], ones_col[:, :], mask_sb[t][:, :],
                                 start=(t == 0), stop=(t == NT - 1))

            deg_own = sb.tile([1, R], F32, name="deg_own", tag="deg_own")
            nc.vector.tensor_copy(deg_own[:, :], pdeg[:, :])

            # AllGather deg (the cross-core degree exchange).
            degb_in = dram.tile([R], F32, name="degb_in", tag="degb_in")
            degb_out = dram.tile([N], F32, addr_space="Shared", name="degb_out", tag="degb_out")
            nc.gpsimd.dma_start(degb_in[:], deg_own[:, :])
            nc.gpsimd.collective_compute(
                "AllGather", mybir.AluOpType.bypass, replica_groups=rg,
                ins=[degb_in.opt()], outs=[degb_out.opt()])

            # dis = deg^-1/2 in partition-major layouts (global and own-rows).
            deg_glob = sb.tile([P, NT], F32, name="deg_glob", tag="deg_glob")
            deg_ownp = sb.tile([P, MT], F32, name="deg_ownp", tag="deg_ownp")
            nc.sync.dma_start(deg_glob[:, :], degb_out.rearrange("(t p) -> p t", p=P))
            nc.sync.dma_start(deg_ownp[:, :], degb_in.rearrange("(t p) -> p t", p=P))
            dis_glob = sb.tile([P, NT], F32, name="dis_glob", tag="dis_glob")
            dis_own = sb.tile([P, MT], F32, name="dis_own", tag="dis_own")
            nc.vector.reciprocal(dis_glob[:, :], deg_glob[:, :])
            nc.scalar.sqrt(dis_glob[:, :], dis_glob[:, :])
            nc.vector.reciprocal(dis_own[:, :], deg_ownp[:, :])
            nc.scalar.sqrt(dis_own[:, :], dis_own[:, :])
            # sqrt(deg) row-vector: cancels the dis_r row scaling for the bias.
            invdis_row = sb.tile([1, R], BF16, name="invdis_row", tag="invdis_row")
            nc.scalar.sqrt(invdis_row[:, :], deg_own[:, :])

            # ---- phase 3: y *= dis; out_rows = dis_r * (A @ y) + b ----------
            # Split the 32 per-tile scalings across ACT and DVE so neither
            # engine serializes ~30us ahead of the final matmul.
            for t in range(NT):
                if t % 2 == 0:
                    nc.scalar.mul(y_sb[t][:, :], y_sb[t][:, :], dis_glob[:, t:t + 1])
                else:
                    nc.vector.tensor_scalar(y_sb[t][:, :], y_sb[t][:, :],
                                            dis_glob[:, t:t + 1], None,
                                            mybir.AluOpType.mult)

            for m in range(MT):
                pf = ps_fin.tile([P, C], F32, name=f"psf{m}", tag="psf")
                for t in range(NT):
                    nc.tensor.matmul(pf[:, :],
                                     mask_sb[t][:, P * m:P * (m + 1)],
                                     y_sb[t][:, :],
                                     start=(t == 0), stop=False)
                # += sqrt(deg_r) (x) bias  — cancels against the dis_r scaling
                nc.tensor.matmul(pf[:, :],
                                 invdis_row[:, P * m:P * (m + 1)],
                                 bias_sb[:, :],
                                 start=False, stop=True)
                ot = sbo.tile([P, C], F32, name=f"outt{m}", tag="outt")
                nc.vector.tensor_scalar(ot[:, :], pf[:, :], dis_own[:, m:m + 1],
                                        None, mybir.AluOpType.mult)
                nc.sync.dma_start(out[P * m:P * (m + 1), :], ot[:, :])

    nc.compile()
    return nc


def _get_nc():
    if "nc" not in _cache:
        _cache["nc"] = _build()
    return _cache["nc"]


def _run(inputs, trace=False, trace_cores=None):
    x = np.asarray(inputs["x"], dtype=np.float32)
    adj_weight = np.asarray(inputs["adj_weight"], dtype=np.float32)
    gcn_weight = np.asarray(inputs["gcn_weight"], dtype=np.float32)
    gcn_bias = np.asarray(inputs["gcn_bias"], dtype=np.float32)

    xT = np.ascontiguousarray(x.T).astype(BF)          # [C, N]
    adjW = adj_weight.astype(BF)
    gcnW = gcn_weight.astype(BF)
    bias = gcn_bias.reshape(1, C).astype(BF)

    in_maps = []
    for i in range(N_CORES):
        in_maps.append({
            "xT": xT,
            "xTs": np.ascontiguousarray(xT[:, R * i:R * (i + 1)]),
            "adjW": adjW,
            "gcnW": gcnW,
            "bias": bias,
        })

    nc = _get_nc()
    res = run_bass_kernel_spmd(nc, in_maps, core_ids=list(range(N_CORES)),
                               trace=trace, trace_cores=trace_cores)
    full = np.concatenate([res.results[i]["out"] for i in range(N_CORES)], axis=0)
    return full, res


def kernel(**inputs):
    full, _ = _run(inputs, trace=False)
    return full
